# revision 17
# baseline (speedup 1.0000x reference)
"""Grouped MLP (MoE expert FFN) Bass kernel for 8 Trainium2 NeuronCores.

Problem: 4096 tokens sorted by expert (8 experts, uneven counts), per-expert
GLU MLP:  h = x @ w1[g]  (-> up|gate, 2*2048 cols);  a = silu(up)*gate;
y = a @ w2[g].

Sharding: feature-sliced expert-uniform ("tensor-parallel over INTER").
Core c owns a 1/8 slice of the INTER dimension of EVERY expert:
  w1 slice: per expert, up cols [c*256:(c+1)*256] + gate cols alike -> [1024, 512]
  w2 slice: per expert, rows  [c*256:(c+1)*256]                     -> [256, 1024]
Every core processes ALL tokens of ALL experts against its slice and emits a
PARTIAL y (fc2 summed over its 256 inter features).  The host sums the 8
partials.  Because each core sees the same per-expert token counts, the single
SPMD program is perfectly uniform with ZERO padding: exactly 512
token-equivalents of matmul work per core (vs 768 for the padded
expert-parallel layout), and each core reads exactly 1/8 of the weight bytes.

Everything runs in bf16 (PSUM accumulation in fp32): same PE rate as fp32r but
half the HBM traffic, which is what the ridge regime needs.

Per-core DMA: weights 12.6MB + x 8MB + y 8MB = ~28.6MB bf16 ~= 80us @ 358GB/s;
matmul roofline 6.44 GFLOP @ 78.6 TF/s = 82us.  Balanced at the ridge.
"""

import sys

try:  # concourse normally comes from the container's PYTHONPATH
    import concourse  # noqa: F401
except ImportError:  # pragma: no cover - fallback for stripped env
    for _p in (
        "/root/.axon_site",
        "/root/.axon_site/_ro/trn_rl_repo",
        "/root/.axon_site/_ro/pypackages",
        "/opt/trn_rl_repo",
    ):
        if _p not in sys.path:
            sys.path.append(_p)

from contextlib import ExitStack

import ml_dtypes
import numpy as np

NUM_TOKENS = 4096
HIDDEN = 1024
INTER = 2048
GROUPS = 8
N_CORES = 8

KC = HIDDEN // 128  # 8 k-blocks for fc1
SLICE = INTER // N_CORES  # 256 inter features per core per expert
KB = SLICE // 128  # 2 k-blocks for fc2
MB = HIDDEN // 128  # 8 output blocks of y
W1COLS = 2 * SLICE  # 512 = up(256)|gate(256) per expert slice
BF16 = ml_dtypes.bfloat16


def _ceil_to(x: int, m: int) -> int:
    return ((x + m - 1) // m) * m


def _chunks(n: int):
    """Split n (multiple of 128) into even chunks of <=512, multiples of 128."""
    u = n // 128
    k = (u + 3) // 4
    out = []
    off = 0
    for i in range(k):
        ui = u // k + (1 if i < u % k else 0)
        out.append((off, ui * 128))
        off += ui * 128
    return out


_PROGRAM_CACHE: dict = {}


def _build_program(counts: tuple):
    """Build + compile the single-core Bass program (same NEFF on all cores).

    counts: per-expert token counts, each a multiple of 128."""
    import concourse.bass as bass  # noqa: F401
    import concourse.mybir as mybir
    import concourse.tile as tile
    from concourse import bacc

    f32 = mybir.dt.float32
    bf16 = mybir.dt.bfloat16
    silu = mybir.ActivationFunctionType.Silu

    tot = sum(counts)
    offs = [0]
    for n in counts:
        offs.append(offs[-1] + n)

    nc = bacc.Bacc("TRN2", target_bir_lowering=False, debug=False)

    # Per-expert staged inputs (host pre-transposed, see _prep_core_inputs):
    #  x_d[g]  : [128, KC * n_g]   xT, col = kc * n_g + t
    #  w1_d[g] : [128, KC * 512]   col = kc*512 + b*128 + m, b in up0,up1,gt0,gt1
    #  w2_d[g] : [128, KB * 1024]  col = kb*1024 + mb*128 + m
    # Variable n_g -> flat x tensor with per-expert column offsets.
    x_d = nc.dram_tensor("xT", [128, KC * tot], bf16, kind="ExternalInput").ap()
    # merged per-expert weights: w1 slice cols [0, KC*512) | w2 slice cols after
    WCOLS = KC * W1COLS + KB * HIDDEN
    w_d = nc.dram_tensor("ws", [GROUPS, 128, WCOLS], bf16, kind="ExternalInput").ap()
    # y_d: partial yT, expert g block at cols [MB*off_g, MB*(off_g+n_g)),
    # within block col = mb * n_g + t
    y_d = nc.dram_tensor("yT", [128, MB * tot], bf16, kind="ExternalOutput").ap()

    with tile.TileContext(nc) as tc, ExitStack() as ctx:
        xp = ctx.enter_context(tc.tile_pool(name="x", bufs=3))
        w1p = ctx.enter_context(tc.tile_pool(name="w1", bufs=3))
        w2p = ctx.enter_context(tc.tile_pool(name="w2", bufs=3))
        hp = ctx.enter_context(tc.tile_pool(name="hgl", bufs=6))
        tp = ctx.enter_context(tc.tile_pool(name="tmp", bufs=4))
        yp = ctx.enter_context(tc.tile_pool(name="y", bufs=3))
        pup = ctx.enter_context(tc.tile_pool(name="pu", bufs=2, space="PSUM"))
        pgp = ctx.enter_context(tc.tile_pool(name="pg", bufs=2, space="PSUM"))
        pyp = ctx.enter_context(tc.tile_pool(name="py", bufs=4, space="PSUM"))

        state = {}  # per-expert tiles alive across the software pipeline

        def load(g, split=False):
            n = counts[g]
            if n == 0:
                return
            x_t = xp.tile([128, KC * n], bf16, tag="x")
            w_t = w1p.tile([128, WCOLS], bf16, tag="w")
            if split:
                # early experts: piecewise DMAs so the first matmul chains can
                # chase the transfers instead of waiting for the whole tile
                for kc in range(KC):
                    nc.sync.dma_start(
                        out=w_t[:, kc * W1COLS : (kc + 1) * W1COLS],
                        in_=w_d[g][:, kc * W1COLS : (kc + 1) * W1COLS],
                    )
                    nc.scalar.dma_start(
                        out=x_t[:, kc * n : (kc + 1) * n],
                        in_=x_d[:, KC * offs[g] + kc * n : KC * offs[g] + (kc + 1) * n],
                    )
                nc.sync.dma_start(
                    out=w_t[:, KC * W1COLS :], in_=w_d[g][:, KC * W1COLS :]
                )
            else:
                nc.scalar.dma_start(
                    out=x_t, in_=x_d[:, KC * offs[g] : KC * offs[g + 1]]
                )
                nc.sync.dma_start(out=w_t, in_=w_d[g])
            state[g] = {"x": x_t, "w": w_t, "hgl": []}

        def fc1(g):
            n = counts[g]
            if n == 0:
                return
            st = state[g]
            x_t, w1t = st["x"], st["w"]
            for off, nl in _chunks(n):
                hgl = hp.tile([128, 2 * nl], bf16, tag="hgl")
                st["hgl"].append((off, nl, hgl))
                # blocks interleaved (up_b, gate_b) so each GLU can start as
                # soon as its pair of psum accumulations is done
                for b in range(KB):
                    pu = pup.tile([128, nl], f32, tag="ps")
                    pg = pgp.tile([128, nl], f32, tag="ps")
                    for ps, blk in ((pu, b), (pg, b + KB)):
                        for kc in range(KC):
                            nc.tensor.matmul(
                                ps,
                                w1t[
                                    :,
                                    kc * W1COLS + blk * 128 : kc * W1COLS + (blk + 1) * 128,
                                ],
                                x_t[:, kc * n + off : kc * n + off + nl],
                                start=(kc == 0),
                                stop=(kc == KC - 1),
                            )
                    # GLU: hgl[b] = silu(up_b) * gate_b, bf16
                    tmp = tp.tile([128, nl], f32, tag="tmp")
                    nc.scalar.activation(tmp, pu, silu)
                    nc.vector.tensor_mul(hgl[:, b * nl : (b + 1) * nl], tmp, pg)

        def fc2(g, tail_g=False):
            n = counts[g]
            if n == 0:
                return
            st = state[g]
            w2t = st["w"]
            W2OFF = KC * W1COLS
            y_t = yp.tile([128, MB * n], bf16, tag="y")
            nchunk = len(st["hgl"])
            for ci, (off, nl, hgl) in enumerate(st["hgl"]):
                for grp in range(0, MB, 4):
                    # kb0 chains for 4 mb first, then kb1: the last GLU block
                    # of fc1(g) finishes under the kb0 group, so emitting
                    # fc2(g) right after fc1(g) never stalls the PE
                    pys = []
                    for mb in range(grp, grp + 4):
                        py = pyp.tile([128, nl], f32, tag="py")
                        pys.append(py)
                        nc.tensor.matmul(
                            py,
                            w2t[:, W2OFF + mb * 128 : W2OFF + (mb + 1) * 128],
                            hgl[:, :nl],
                            start=True,
                            stop=False,
                        )
                    for i, mb in enumerate(range(grp, grp + 4)):
                        nc.tensor.matmul(
                            pys[i],
                            w2t[:, W2OFF + HIDDEN + mb * 128 : W2OFF + HIDDEN + (mb + 1) * 128],
                            hgl[:, nl : 2 * nl],
                            start=False,
                            stop=True,
                        )
                        dst = y_t[:, mb * n + off : mb * n + off + nl]
                        # split psum->bf16 casts across DVE and ACT so neither
                        # engine serializes the GLU or the kernel tail
                        if mb % 2 == 1:
                            nc.scalar.copy(out=dst, in_=pys[i])
                        else:
                            nc.vector.tensor_copy(out=dst, in_=pys[i])
                        # stream y out in halves (after casts mb3 / mb7) so
                        # the final DMA mostly drains during compute and the
                        # Sync queue sees few y descriptors
                        if ci == nchunk - 1:
                            step = 2 if (tail_g and mb >= 4) else 4
                            if (mb + 1) % step == 0 and (step == 4 or mb >= 5):
                                lo = mb + 1 - step
                                nc.sync.dma_start(
                                    out=y_d[
                                        :,
                                        MB * offs[g] + lo * n : MB * offs[g] + (mb + 1) * n,
                                    ],
                                    in_=y_t[:, lo * n : (mb + 1) * n],
                                )
            del state[g]

        # schedule: fc2(g) directly follows fc1(g) (kb-grouped emission in
        # fc2 hides the last GLU latency), so the PE always has resident work
        # queued while the next expert's weights/tokens stream in.
        # Order: big experts early (compute surplus covers the DMA ramp),
        # the two smallest last (short y-drain tail).
        by_size = sorted(range(GROUPS), key=lambda g: counts[g])
        tail2 = by_size[:2][::-1]  # second-smallest, then smallest
        mids = sorted(
            (g for g in range(GROUPS) if g not in tail2),
            key=lambda g: -counts[g],
        )
        order = [mids[1]] + [mids[0]] + mids[2:] + tail2
        load(order[0], split=True)
        load(order[1])
        for i, g in enumerate(order):
            if i + 2 < len(order):
                load(order[i + 2])
            fc1(g)
            fc2(g, tail_g=(i == len(order) - 1))

    nc.compile()
    return nc


def _get_program(counts: tuple):
    if counts not in _PROGRAM_CACHE:
        _PROGRAM_CACHE[counts] = _build_program(counts)
    return _PROGRAM_CACHE[counts]


def _prep_inputs(x, w1, w2, counts, counts_pad):
    """Host-side staging: shared xT + per-core weight slices (all bf16)."""
    tot = int(sum(counts_pad))
    offs = np.zeros(GROUPS + 1, np.int64)
    offs[1:] = np.cumsum(counts_pad)
    roffs = np.zeros(GROUPS + 1, np.int64)
    roffs[1:] = np.cumsum(counts)

    # xT: [128, KC * tot]; expert g at cols [KC*offs[g] ...), col = kc*n + t
    xT = np.zeros((128, KC * tot), BF16)
    xTfull = np.ascontiguousarray(x.T).astype(BF16)  # [1024, 4096real]
    for g in range(GROUPS):
        n, npad = int(counts[g]), int(counts_pad[g])
        if npad == 0:
            continue
        seg = np.zeros((KC, 128, npad), BF16)
        if n:
            seg[:, :, :n] = xTfull[:, roffs[g] : roffs[g + 1]].reshape(KC, 128, n)
        xT[:, KC * offs[g] : KC * offs[g + 1]] = seg.transpose(1, 0, 2).reshape(128, -1)

    in_maps = []
    for c in range(N_CORES):
        # w1 slice: up cols [c*256,(c+1)*256) and gate cols 2048 + same
        up = w1[:, :, c * SLICE : (c + 1) * SLICE]  # [G, 1024, 256]
        gt = w1[:, :, INTER + c * SLICE : INTER + (c + 1) * SLICE]
        w1s = np.concatenate([up, gt], axis=2)  # [G, 1024, 512]
        w1s = (
            w1s.reshape(GROUPS, KC, 128, W1COLS)
            .transpose(0, 2, 1, 3)
            .reshape(GROUPS, 128, KC * W1COLS)
            .astype(BF16)
        )
        w2s = w2[:, c * SLICE : (c + 1) * SLICE, :]  # [G, 256, 1024]
        w2s = (
            w2s.reshape(GROUPS, KB, 128, HIDDEN)
            .transpose(0, 2, 1, 3)
            .reshape(GROUPS, 128, KB * HIDDEN)
            .astype(BF16)
        )
        ws = np.concatenate([w1s, w2s], axis=2)  # [G, 128, 6144]
        in_maps.append({"xT": xT, "ws": np.ascontiguousarray(ws)})
    return in_maps, offs, roffs


_LAST_RESULTS = {}  # exposed for test.py (exec time, trace paths)


def kernel(permuted_tokens, tokens_per_expert, w1, w2, _trace=False):
    from concourse.bass_utils import run_bass_kernel_spmd

    x = np.asarray(permuted_tokens, np.float32)
    counts = np.asarray(tokens_per_expert, np.int64)
    w1 = np.asarray(w1, np.float32)
    w2 = np.asarray(w2, np.float32)

    counts_pad = np.array([_ceil_to(int(n), 128) for n in counts], np.int64)
    nc = _get_program(tuple(int(n) for n in counts_pad))

    in_maps, offs, roffs = _prep_inputs(x, w1, w2, counts, counts_pad)

    kwargs = {}
    if _trace:
        kwargs = dict(trace=True, trace_cores=list(range(N_CORES)))
    res = run_bass_kernel_spmd(nc, in_maps, core_ids=list(range(N_CORES)), **kwargs)
    _LAST_RESULTS["res"] = res

    # host reduction: sum the 8 partial yT, then decode layout
    tot = int(counts_pad.sum())
    acc = np.zeros((128, MB * tot), np.float32)
    for c in range(N_CORES):
        acc += np.asarray(res.results[c]["yT"], np.float32)

    out = np.empty((x.shape[0], HIDDEN), np.float32)
    for g in range(GROUPS):
        n, npad = int(counts[g]), int(counts_pad[g])
        if n == 0:
            continue
        blk = acc[:, MB * offs[g] : MB * offs[g + 1]].reshape(128, MB, npad)
        # blk[p, mb, t] = y[roffs[g]+t, mb*128+p]
        out[roffs[g] : roffs[g + 1]] = (
            blk[:, :, :n].transpose(2, 1, 0).reshape(n, HIDDEN)
        )
    return out


# revision 18
# speedup vs baseline: 1.0516x; 1.0516x over previous
"""Grouped MLP (MoE expert FFN) Bass kernel for 8 Trainium2 NeuronCores.

Problem: 4096 tokens sorted by expert (8 experts, uneven counts), per-expert
GLU MLP:  h = x @ w1[g]  (-> up|gate, 2*2048 cols);  a = silu(up)*gate;
y = a @ w2[g].

Sharding: feature-sliced expert-uniform ("tensor-parallel over INTER").
Core c owns a 1/8 slice of the INTER dimension of EVERY expert:
  w1 slice: per expert, up cols [c*256:(c+1)*256] + gate cols alike -> [1024, 512]
  w2 slice: per expert, rows  [c*256:(c+1)*256]                     -> [256, 1024]
Every core processes ALL tokens of ALL experts against its slice and emits a
PARTIAL y (fc2 summed over its 256 inter features).  The host sums the 8
partials.  Because each core sees the same per-expert token counts, the single
SPMD program is perfectly uniform with ZERO padding: exactly 512
token-equivalents of matmul work per core (vs 768 for the padded
expert-parallel layout), and each core reads exactly 1/8 of the weight bytes.

Everything runs in bf16 (PSUM accumulation in fp32): same PE rate as fp32r but
half the HBM traffic, which is what the ridge regime needs.

Per-core DMA: weights 12.6MB + x 8MB + y 8MB = ~28.6MB bf16 ~= 80us @ 358GB/s;
matmul roofline 6.44 GFLOP @ 78.6 TF/s = 82us.  Balanced at the ridge.
"""

import sys

try:  # concourse normally comes from the container's PYTHONPATH
    import concourse  # noqa: F401
except ImportError:  # pragma: no cover - fallback for stripped env
    for _p in (
        "/root/.axon_site",
        "/root/.axon_site/_ro/trn_rl_repo",
        "/root/.axon_site/_ro/pypackages",
        "/opt/trn_rl_repo",
    ):
        if _p not in sys.path:
            sys.path.append(_p)

from contextlib import ExitStack

import ml_dtypes
import numpy as np

NUM_TOKENS = 4096
HIDDEN = 1024
INTER = 2048
GROUPS = 8
N_CORES = 8

KC = HIDDEN // 128  # 8 k-blocks for fc1
SLICE = INTER // N_CORES  # 256 inter features per core per expert
KB = SLICE // 128  # 2 k-blocks for fc2
MB = HIDDEN // 128  # 8 output blocks of y
W1COLS = 2 * SLICE  # 512 = up(256)|gate(256) per expert slice
BF16 = ml_dtypes.bfloat16


def _ceil_to(x: int, m: int) -> int:
    return ((x + m - 1) // m) * m


def _chunks(n: int):
    """Split n (multiple of 128) into even chunks of <=512, multiples of 128."""
    u = n // 128
    k = (u + 3) // 4
    out = []
    off = 0
    for i in range(k):
        ui = u // k + (1 if i < u % k else 0)
        out.append((off, ui * 128))
        off += ui * 128
    return out


_PROGRAM_CACHE: dict = {}


def _build_program(counts: tuple):
    """Build + compile the single-core Bass program (same NEFF on all cores).

    counts: per-expert token counts, each a multiple of 128."""
    import concourse.bass as bass  # noqa: F401
    import concourse.mybir as mybir
    import concourse.tile as tile
    from concourse import bacc

    f32 = mybir.dt.float32
    bf16 = mybir.dt.bfloat16
    silu = mybir.ActivationFunctionType.Silu

    tot = sum(counts)
    offs = [0]
    for n in counts:
        offs.append(offs[-1] + n)

    nc = bacc.Bacc("TRN2", target_bir_lowering=False, debug=False)

    # Per-expert staged inputs (host pre-transposed, see _prep_core_inputs):
    #  x_d[g]  : [128, KC * n_g]   xT, col = kc * n_g + t
    #  w1_d[g] : [128, KC * 512]   col = kc*512 + b*128 + m, b in up0,up1,gt0,gt1
    #  w2_d[g] : [128, KB * 1024]  col = kb*1024 + mb*128 + m
    # Variable n_g -> flat x tensor with per-expert column offsets.
    x_d = nc.dram_tensor("xT", [128, KC * tot], bf16, kind="ExternalInput").ap()
    # merged per-expert weights: w1 slice cols [0, KC*512) | w2 slice cols after
    WCOLS = KC * W1COLS + KB * HIDDEN
    w_d = nc.dram_tensor("ws", [GROUPS, 128, WCOLS], bf16, kind="ExternalInput").ap()
    # y_d: partial yT, expert g block at cols [MB*off_g, MB*(off_g+n_g)),
    # within block col = mb * n_g + t
    y_d = nc.dram_tensor("yT", [128, MB * tot], bf16, kind="ExternalOutput").ap()

    with tile.TileContext(nc) as tc, ExitStack() as ctx:
        xp = ctx.enter_context(tc.tile_pool(name="x", bufs=3))
        w1p = ctx.enter_context(tc.tile_pool(name="w1", bufs=3))
        w2p = ctx.enter_context(tc.tile_pool(name="w2", bufs=3))
        hp = ctx.enter_context(tc.tile_pool(name="hgl", bufs=6))
        tp = ctx.enter_context(tc.tile_pool(name="tmp", bufs=4))
        yp = ctx.enter_context(tc.tile_pool(name="y", bufs=3))
        pup = ctx.enter_context(tc.tile_pool(name="pu", bufs=2, space="PSUM"))
        pgp = ctx.enter_context(tc.tile_pool(name="pg", bufs=2, space="PSUM"))
        pyp = ctx.enter_context(tc.tile_pool(name="py", bufs=4, space="PSUM"))

        state = {}  # per-expert tiles alive across the software pipeline

        def load(g, split=False):
            n = counts[g]
            if n == 0:
                return
            x_t = xp.tile([128, KC * n], bf16, tag="x")
            w_t = w1p.tile([128, WCOLS], bf16, tag="w")
            if split:
                # early experts: piecewise DMAs so the first matmul chains can
                # chase the transfers instead of waiting for the whole tile
                for kc in range(KC):
                    nc.sync.dma_start(
                        out=w_t[:, kc * W1COLS : (kc + 1) * W1COLS],
                        in_=w_d[g][:, kc * W1COLS : (kc + 1) * W1COLS],
                    )
                    nc.sync.dma_start(
                        out=x_t[:, kc * n : (kc + 1) * n],
                        in_=x_d[:, KC * offs[g] + kc * n : KC * offs[g] + (kc + 1) * n],
                    )
                nc.sync.dma_start(
                    out=w_t[:, KC * W1COLS :], in_=w_d[g][:, KC * W1COLS :]
                )
            else:
                nc.sync.dma_start(
                    out=x_t, in_=x_d[:, KC * offs[g] : KC * offs[g + 1]]
                )
                nc.sync.dma_start(out=w_t, in_=w_d[g])
            state[g] = {"x": x_t, "w": w_t, "hgl": []}

        def fc1(g):
            n = counts[g]
            if n == 0:
                return
            st = state[g]
            x_t, w1t = st["x"], st["w"]
            for off, nl in _chunks(n):
                hgl = hp.tile([128, 2 * nl], bf16, tag="hgl")
                st["hgl"].append((off, nl, hgl))
                # blocks interleaved (up_b, gate_b) so each GLU can start as
                # soon as its pair of psum accumulations is done
                for b in range(KB):
                    pu = pup.tile([128, nl], f32, tag="ps")
                    pg = pgp.tile([128, nl], f32, tag="ps")
                    for ps, blk in ((pu, b), (pg, b + KB)):
                        for kc in range(KC):
                            nc.tensor.matmul(
                                ps,
                                w1t[
                                    :,
                                    kc * W1COLS + blk * 128 : kc * W1COLS + (blk + 1) * 128,
                                ],
                                x_t[:, kc * n + off : kc * n + off + nl],
                                start=(kc == 0),
                                stop=(kc == KC - 1),
                            )
                    # GLU: hgl[b] = silu(up_b) * gate_b, bf16
                    tmp = tp.tile([128, nl], f32, tag="tmp")
                    nc.scalar.activation(tmp, pu, silu)
                    nc.vector.tensor_mul(hgl[:, b * nl : (b + 1) * nl], tmp, pg)

        def fc2(g, tail_g=False):
            n = counts[g]
            if n == 0:
                return
            st = state[g]
            w2t = st["w"]
            W2OFF = KC * W1COLS
            y_t = yp.tile([128, MB * n], bf16, tag="y")
            nchunk = len(st["hgl"])
            for ci, (off, nl, hgl) in enumerate(st["hgl"]):
                for grp in range(0, MB, 4):
                    # kb0 chains for 4 mb first, then kb1: the last GLU block
                    # of fc1(g) finishes under the kb0 group, so emitting
                    # fc2(g) right after fc1(g) never stalls the PE
                    pys = []
                    for mb in range(grp, grp + 4):
                        py = pyp.tile([128, nl], f32, tag="py")
                        pys.append(py)
                        nc.tensor.matmul(
                            py,
                            w2t[:, W2OFF + mb * 128 : W2OFF + (mb + 1) * 128],
                            hgl[:, :nl],
                            start=True,
                            stop=False,
                        )
                    for i, mb in enumerate(range(grp, grp + 4)):
                        nc.tensor.matmul(
                            pys[i],
                            w2t[:, W2OFF + HIDDEN + mb * 128 : W2OFF + HIDDEN + (mb + 1) * 128],
                            hgl[:, nl : 2 * nl],
                            start=False,
                            stop=True,
                        )
                        dst = y_t[:, mb * n + off : mb * n + off + nl]
                        # split psum->bf16 casts across DVE and ACT so neither
                        # engine serializes the GLU or the kernel tail
                        if mb % 2 == 1:
                            nc.scalar.copy(out=dst, in_=pys[i])
                        else:
                            nc.vector.tensor_copy(out=dst, in_=pys[i])
                        # stream y out in halves (after casts mb3 / mb7) so
                        # the final DMA mostly drains during compute and the
                        # Sync queue sees few y descriptors
                        if ci == nchunk - 1:
                            step = 2 if (tail_g and mb >= 4) else 4
                            if (mb + 1) % step == 0 and (step == 4 or mb >= 5):
                                lo = mb + 1 - step
                                nc.sync.dma_start(
                                    out=y_d[
                                        :,
                                        MB * offs[g] + lo * n : MB * offs[g] + (mb + 1) * n,
                                    ],
                                    in_=y_t[:, lo * n : (mb + 1) * n],
                                )
            del state[g]

        # schedule: fc2(g) directly follows fc1(g) (kb-grouped emission in
        # fc2 hides the last GLU latency), so the PE always has resident work
        # queued while the next expert's weights/tokens stream in.
        # Order: big experts early (compute surplus covers the DMA ramp),
        # the two smallest last (short y-drain tail).
        by_size = sorted(range(GROUPS), key=lambda g: counts[g])
        tail2 = by_size[:2][::-1]  # second-smallest, then smallest
        mids = sorted(
            (g for g in range(GROUPS) if g not in tail2),
            key=lambda g: -counts[g],
        )
        order = [mids[1]] + [mids[0]] + mids[2:] + tail2
        load(order[0], split=True)
        load(order[1])
        for i, g in enumerate(order):
            if i + 2 < len(order):
                load(order[i + 2])
            fc1(g)
            fc2(g, tail_g=(i == len(order) - 1))

    nc.compile()
    return nc


def _get_program(counts: tuple):
    if counts not in _PROGRAM_CACHE:
        _PROGRAM_CACHE[counts] = _build_program(counts)
    return _PROGRAM_CACHE[counts]


def _prep_inputs(x, w1, w2, counts, counts_pad):
    """Host-side staging: shared xT + per-core weight slices (all bf16)."""
    tot = int(sum(counts_pad))
    offs = np.zeros(GROUPS + 1, np.int64)
    offs[1:] = np.cumsum(counts_pad)
    roffs = np.zeros(GROUPS + 1, np.int64)
    roffs[1:] = np.cumsum(counts)

    # xT: [128, KC * tot]; expert g at cols [KC*offs[g] ...), col = kc*n + t
    xT = np.zeros((128, KC * tot), BF16)
    xTfull = np.ascontiguousarray(x.T).astype(BF16)  # [1024, 4096real]
    for g in range(GROUPS):
        n, npad = int(counts[g]), int(counts_pad[g])
        if npad == 0:
            continue
        seg = np.zeros((KC, 128, npad), BF16)
        if n:
            seg[:, :, :n] = xTfull[:, roffs[g] : roffs[g + 1]].reshape(KC, 128, n)
        xT[:, KC * offs[g] : KC * offs[g + 1]] = seg.transpose(1, 0, 2).reshape(128, -1)

    in_maps = []
    for c in range(N_CORES):
        # w1 slice: up cols [c*256,(c+1)*256) and gate cols 2048 + same
        up = w1[:, :, c * SLICE : (c + 1) * SLICE]  # [G, 1024, 256]
        gt = w1[:, :, INTER + c * SLICE : INTER + (c + 1) * SLICE]
        w1s = np.concatenate([up, gt], axis=2)  # [G, 1024, 512]
        w1s = (
            w1s.reshape(GROUPS, KC, 128, W1COLS)
            .transpose(0, 2, 1, 3)
            .reshape(GROUPS, 128, KC * W1COLS)
            .astype(BF16)
        )
        w2s = w2[:, c * SLICE : (c + 1) * SLICE, :]  # [G, 256, 1024]
        w2s = (
            w2s.reshape(GROUPS, KB, 128, HIDDEN)
            .transpose(0, 2, 1, 3)
            .reshape(GROUPS, 128, KB * HIDDEN)
            .astype(BF16)
        )
        ws = np.concatenate([w1s, w2s], axis=2)  # [G, 128, 6144]
        in_maps.append({"xT": xT, "ws": np.ascontiguousarray(ws)})
    return in_maps, offs, roffs


_LAST_RESULTS = {}  # exposed for test.py (exec time, trace paths)


def kernel(permuted_tokens, tokens_per_expert, w1, w2, _trace=False):
    from concourse.bass_utils import run_bass_kernel_spmd

    x = np.asarray(permuted_tokens, np.float32)
    counts = np.asarray(tokens_per_expert, np.int64)
    w1 = np.asarray(w1, np.float32)
    w2 = np.asarray(w2, np.float32)

    counts_pad = np.array([_ceil_to(int(n), 128) for n in counts], np.int64)
    nc = _get_program(tuple(int(n) for n in counts_pad))

    in_maps, offs, roffs = _prep_inputs(x, w1, w2, counts, counts_pad)

    kwargs = {}
    if _trace:
        kwargs = dict(trace=True, trace_cores=list(range(N_CORES)))
    res = run_bass_kernel_spmd(nc, in_maps, core_ids=list(range(N_CORES)), **kwargs)
    _LAST_RESULTS["res"] = res

    # host reduction: sum the 8 partial yT, then decode layout
    tot = int(counts_pad.sum())
    acc = np.zeros((128, MB * tot), np.float32)
    for c in range(N_CORES):
        acc += np.asarray(res.results[c]["yT"], np.float32)

    out = np.empty((x.shape[0], HIDDEN), np.float32)
    for g in range(GROUPS):
        n, npad = int(counts[g]), int(counts_pad[g])
        if n == 0:
            continue
        blk = acc[:, MB * offs[g] : MB * offs[g + 1]].reshape(128, MB, npad)
        # blk[p, mb, t] = y[roffs[g]+t, mb*128+p]
        out[roffs[g] : roffs[g + 1]] = (
            blk[:, :, :n].transpose(2, 1, 0).reshape(n, HIDDEN)
        )
    return out


# revision 19
# speedup vs baseline: 1.0545x; 1.0028x over previous
"""Grouped MLP (MoE expert FFN) Bass kernel for 8 Trainium2 NeuronCores.

Problem: 4096 tokens sorted by expert (8 experts, uneven counts), per-expert
GLU MLP:  h = x @ w1[g]  (-> up|gate, 2*2048 cols);  a = silu(up)*gate;
y = a @ w2[g].

Sharding: feature-sliced expert-uniform ("tensor-parallel over INTER").
Core c owns a 1/8 slice of the INTER dimension of EVERY expert:
  w1 slice: per expert, up cols [c*256:(c+1)*256] + gate cols alike -> [1024, 512]
  w2 slice: per expert, rows  [c*256:(c+1)*256]                     -> [256, 1024]
Every core processes ALL tokens of ALL experts against its slice and emits a
PARTIAL y (fc2 summed over its 256 inter features).  The host sums the 8
partials.  Because each core sees the same per-expert token counts, the single
SPMD program is perfectly uniform with ZERO padding: exactly 512
token-equivalents of matmul work per core (vs 768 for the padded
expert-parallel layout), and each core reads exactly 1/8 of the weight bytes.

Everything runs in bf16 (PSUM accumulation in fp32): same PE rate as fp32r but
half the HBM traffic, which is what the ridge regime needs.

Per-core DMA: weights 12.6MB + x 8MB + y 8MB = ~28.6MB bf16 ~= 80us @ 358GB/s;
matmul roofline 6.44 GFLOP @ 78.6 TF/s = 82us.  Balanced at the ridge.
"""

import sys

try:  # concourse normally comes from the container's PYTHONPATH
    import concourse  # noqa: F401
except ImportError:  # pragma: no cover - fallback for stripped env
    for _p in (
        "/root/.axon_site",
        "/root/.axon_site/_ro/trn_rl_repo",
        "/root/.axon_site/_ro/pypackages",
        "/opt/trn_rl_repo",
    ):
        if _p not in sys.path:
            sys.path.append(_p)

from contextlib import ExitStack

import ml_dtypes
import numpy as np

NUM_TOKENS = 4096
HIDDEN = 1024
INTER = 2048
GROUPS = 8
N_CORES = 8

KC = HIDDEN // 128  # 8 k-blocks for fc1
SLICE = INTER // N_CORES  # 256 inter features per core per expert
KB = SLICE // 128  # 2 k-blocks for fc2
MB = HIDDEN // 128  # 8 output blocks of y
W1COLS = 2 * SLICE  # 512 = up(256)|gate(256) per expert slice
BF16 = ml_dtypes.bfloat16


def _ceil_to(x: int, m: int) -> int:
    return ((x + m - 1) // m) * m


def _chunks(n: int):
    """Split n (multiple of 128) into even chunks of <=512, multiples of 128."""
    u = n // 128
    k = (u + 3) // 4
    out = []
    off = 0
    for i in range(k):
        ui = u // k + (1 if i < u % k else 0)
        out.append((off, ui * 128))
        off += ui * 128
    return out


_PROGRAM_CACHE: dict = {}


def _build_program(counts: tuple):
    """Build + compile the single-core Bass program (same NEFF on all cores).

    counts: per-expert token counts, each a multiple of 128."""
    import concourse.bass as bass  # noqa: F401
    import concourse.mybir as mybir
    import concourse.tile as tile
    from concourse import bacc

    f32 = mybir.dt.float32
    bf16 = mybir.dt.bfloat16
    silu = mybir.ActivationFunctionType.Silu

    tot = sum(counts)
    offs = [0]
    for n in counts:
        offs.append(offs[-1] + n)

    nc = bacc.Bacc("TRN2", target_bir_lowering=False, debug=False)

    # Per-expert staged inputs (host pre-transposed, see _prep_core_inputs):
    #  x_d[g]  : [128, KC * n_g]   xT, col = kc * n_g + t
    #  w1_d[g] : [128, KC * 512]   col = kc*512 + b*128 + m, b in up0,up1,gt0,gt1
    #  w2_d[g] : [128, KB * 1024]  col = kb*1024 + mb*128 + m
    # Variable n_g -> flat x tensor with per-expert column offsets.
    x_d = nc.dram_tensor("xT", [128, KC * tot], bf16, kind="ExternalInput").ap()
    # merged per-expert weights: w1 slice cols [0, KC*512) | w2 slice cols after
    WCOLS = KC * W1COLS + KB * HIDDEN
    w_d = nc.dram_tensor("ws", [GROUPS, 128, WCOLS], bf16, kind="ExternalInput").ap()
    # y_d: partial yT, expert g block at cols [MB*off_g, MB*(off_g+n_g)),
    # within block col = mb * n_g + t
    y_d = nc.dram_tensor("yT", [128, MB * tot], bf16, kind="ExternalOutput").ap()

    with tile.TileContext(nc) as tc, ExitStack() as ctx:
        xp = ctx.enter_context(tc.tile_pool(name="x", bufs=4))
        w1p = ctx.enter_context(tc.tile_pool(name="w1", bufs=4))
        w2p = ctx.enter_context(tc.tile_pool(name="w2", bufs=3))
        hp = ctx.enter_context(tc.tile_pool(name="hgl", bufs=6))
        tp = ctx.enter_context(tc.tile_pool(name="tmp", bufs=4))
        yp = ctx.enter_context(tc.tile_pool(name="y", bufs=3))
        pup = ctx.enter_context(tc.tile_pool(name="pu", bufs=2, space="PSUM"))
        pgp = ctx.enter_context(tc.tile_pool(name="pg", bufs=2, space="PSUM"))
        pyp = ctx.enter_context(tc.tile_pool(name="py", bufs=4, space="PSUM"))

        state = {}  # per-expert tiles alive across the software pipeline

        def load(g, split=False):
            n = counts[g]
            if n == 0:
                return
            x_t = xp.tile([128, KC * n], bf16, tag="x")
            w_t = w1p.tile([128, WCOLS], bf16, tag="w")
            if split:
                # first expert: piecewise DMAs (2 k-blocks per piece) so the
                # first matmul chains chase the transfers; coarse enough that
                # the ~0.6us/descriptor Sync issue time stays small
                for kc in range(0, KC, 2):
                    nc.sync.dma_start(
                        out=w_t[:, kc * W1COLS : (kc + 2) * W1COLS],
                        in_=w_d[g][:, kc * W1COLS : (kc + 2) * W1COLS],
                    )
                    nc.sync.dma_start(
                        out=x_t[:, kc * n : (kc + 2) * n],
                        in_=x_d[:, KC * offs[g] + kc * n : KC * offs[g] + (kc + 2) * n],
                    )
                nc.sync.dma_start(
                    out=w_t[:, KC * W1COLS :], in_=w_d[g][:, KC * W1COLS :]
                )
            else:
                nc.sync.dma_start(
                    out=x_t, in_=x_d[:, KC * offs[g] : KC * offs[g + 1]]
                )
                nc.sync.dma_start(out=w_t, in_=w_d[g])
            state[g] = {"x": x_t, "w": w_t, "hgl": []}

        def fc1(g):
            n = counts[g]
            if n == 0:
                return
            st = state[g]
            x_t, w1t = st["x"], st["w"]
            for off, nl in _chunks(n):
                hgl = hp.tile([128, 2 * nl], bf16, tag="hgl")
                st["hgl"].append((off, nl, hgl))
                # blocks interleaved (up_b, gate_b) so each GLU can start as
                # soon as its pair of psum accumulations is done
                for b in range(KB):
                    pu = pup.tile([128, nl], f32, tag="ps")
                    pg = pgp.tile([128, nl], f32, tag="ps")
                    for ps, blk in ((pu, b), (pg, b + KB)):
                        for kc in range(KC):
                            nc.tensor.matmul(
                                ps,
                                w1t[
                                    :,
                                    kc * W1COLS + blk * 128 : kc * W1COLS + (blk + 1) * 128,
                                ],
                                x_t[:, kc * n + off : kc * n + off + nl],
                                start=(kc == 0),
                                stop=(kc == KC - 1),
                            )
                    # GLU: hgl[b] = silu(up_b) * gate_b, bf16
                    tmp = tp.tile([128, nl], f32, tag="tmp")
                    nc.scalar.activation(tmp, pu, silu)
                    nc.vector.tensor_mul(hgl[:, b * nl : (b + 1) * nl], tmp, pg)

        def fc2(g, tail_g=False):
            n = counts[g]
            if n == 0:
                return
            st = state[g]
            w2t = st["w"]
            W2OFF = KC * W1COLS
            y_t = yp.tile([128, MB * n], bf16, tag="y")
            nchunk = len(st["hgl"])
            for ci, (off, nl, hgl) in enumerate(st["hgl"]):
                for grp in range(0, MB, 4):
                    # kb0 chains for 4 mb first, then kb1: the last GLU block
                    # of fc1(g) finishes under the kb0 group, so emitting
                    # fc2(g) right after fc1(g) never stalls the PE
                    pys = []
                    for mb in range(grp, grp + 4):
                        py = pyp.tile([128, nl], f32, tag="py")
                        pys.append(py)
                        nc.tensor.matmul(
                            py,
                            w2t[:, W2OFF + mb * 128 : W2OFF + (mb + 1) * 128],
                            hgl[:, :nl],
                            start=True,
                            stop=False,
                        )
                    for i, mb in enumerate(range(grp, grp + 4)):
                        nc.tensor.matmul(
                            pys[i],
                            w2t[:, W2OFF + HIDDEN + mb * 128 : W2OFF + HIDDEN + (mb + 1) * 128],
                            hgl[:, nl : 2 * nl],
                            start=False,
                            stop=True,
                        )
                        dst = y_t[:, mb * n + off : mb * n + off + nl]
                        # split psum->bf16 casts across DVE and ACT so neither
                        # engine serializes the GLU or the kernel tail
                        if mb % 2 == 1:
                            nc.scalar.copy(out=dst, in_=pys[i])
                        else:
                            nc.vector.tensor_copy(out=dst, in_=pys[i])
                        # stream y out in halves (after casts mb3 / mb7) so
                        # the final DMA mostly drains during compute and the
                        # Sync queue sees few y descriptors
                        if ci == nchunk - 1:
                            step = 2 if (tail_g and mb >= 4) else 4
                            if (mb + 1) % step == 0 and (step == 4 or mb >= 5):
                                lo = mb + 1 - step
                                nc.sync.dma_start(
                                    out=y_d[
                                        :,
                                        MB * offs[g] + lo * n : MB * offs[g] + (mb + 1) * n,
                                    ],
                                    in_=y_t[:, lo * n : (mb + 1) * n],
                                )
            del state[g]

        # schedule: fc2(g) directly follows fc1(g) (kb-grouped emission in
        # fc2 hides the last GLU latency), so the PE always has resident work
        # queued while the next expert's weights/tokens stream in.
        # Order: big experts early (compute surplus covers the DMA ramp),
        # the two smallest last (short y-drain tail).
        by_size = sorted(range(GROUPS), key=lambda g: counts[g])
        tail2 = by_size[:2][::-1]  # second-smallest, then smallest
        mids = sorted(
            (g for g in range(GROUPS) if g not in tail2),
            key=lambda g: -counts[g],
        )
        order = [mids[1]] + [mids[0]] + mids[2:] + tail2
        load(order[0], split=True)
        load(order[1])
        load(order[2])
        for i, g in enumerate(order):
            if i + 3 < len(order):
                load(order[i + 3])
            fc1(g)
            fc2(g, tail_g=(i == len(order) - 1))

    nc.compile()
    return nc


def _get_program(counts: tuple):
    if counts not in _PROGRAM_CACHE:
        _PROGRAM_CACHE[counts] = _build_program(counts)
    return _PROGRAM_CACHE[counts]


def _prep_inputs(x, w1, w2, counts, counts_pad):
    """Host-side staging: shared xT + per-core weight slices (all bf16)."""
    tot = int(sum(counts_pad))
    offs = np.zeros(GROUPS + 1, np.int64)
    offs[1:] = np.cumsum(counts_pad)
    roffs = np.zeros(GROUPS + 1, np.int64)
    roffs[1:] = np.cumsum(counts)

    # xT: [128, KC * tot]; expert g at cols [KC*offs[g] ...), col = kc*n + t
    xT = np.zeros((128, KC * tot), BF16)
    xTfull = np.ascontiguousarray(x.T).astype(BF16)  # [1024, 4096real]
    for g in range(GROUPS):
        n, npad = int(counts[g]), int(counts_pad[g])
        if npad == 0:
            continue
        seg = np.zeros((KC, 128, npad), BF16)
        if n:
            seg[:, :, :n] = xTfull[:, roffs[g] : roffs[g + 1]].reshape(KC, 128, n)
        xT[:, KC * offs[g] : KC * offs[g + 1]] = seg.transpose(1, 0, 2).reshape(128, -1)

    in_maps = []
    for c in range(N_CORES):
        # w1 slice: up cols [c*256,(c+1)*256) and gate cols 2048 + same
        up = w1[:, :, c * SLICE : (c + 1) * SLICE]  # [G, 1024, 256]
        gt = w1[:, :, INTER + c * SLICE : INTER + (c + 1) * SLICE]
        w1s = np.concatenate([up, gt], axis=2)  # [G, 1024, 512]
        w1s = (
            w1s.reshape(GROUPS, KC, 128, W1COLS)
            .transpose(0, 2, 1, 3)
            .reshape(GROUPS, 128, KC * W1COLS)
            .astype(BF16)
        )
        w2s = w2[:, c * SLICE : (c + 1) * SLICE, :]  # [G, 256, 1024]
        w2s = (
            w2s.reshape(GROUPS, KB, 128, HIDDEN)
            .transpose(0, 2, 1, 3)
            .reshape(GROUPS, 128, KB * HIDDEN)
            .astype(BF16)
        )
        ws = np.concatenate([w1s, w2s], axis=2)  # [G, 128, 6144]
        in_maps.append({"xT": xT, "ws": np.ascontiguousarray(ws)})
    return in_maps, offs, roffs


_LAST_RESULTS = {}  # exposed for test.py (exec time, trace paths)


def kernel(permuted_tokens, tokens_per_expert, w1, w2, _trace=False):
    from concourse.bass_utils import run_bass_kernel_spmd

    x = np.asarray(permuted_tokens, np.float32)
    counts = np.asarray(tokens_per_expert, np.int64)
    w1 = np.asarray(w1, np.float32)
    w2 = np.asarray(w2, np.float32)

    counts_pad = np.array([_ceil_to(int(n), 128) for n in counts], np.int64)
    nc = _get_program(tuple(int(n) for n in counts_pad))

    in_maps, offs, roffs = _prep_inputs(x, w1, w2, counts, counts_pad)

    kwargs = {}
    if _trace:
        kwargs = dict(trace=True, trace_cores=list(range(N_CORES)))
    res = run_bass_kernel_spmd(nc, in_maps, core_ids=list(range(N_CORES)), **kwargs)
    _LAST_RESULTS["res"] = res

    # host reduction: sum the 8 partial yT, then decode layout
    tot = int(counts_pad.sum())
    acc = np.zeros((128, MB * tot), np.float32)
    for c in range(N_CORES):
        acc += np.asarray(res.results[c]["yT"], np.float32)

    out = np.empty((x.shape[0], HIDDEN), np.float32)
    for g in range(GROUPS):
        n, npad = int(counts[g]), int(counts_pad[g])
        if n == 0:
            continue
        blk = acc[:, MB * offs[g] : MB * offs[g + 1]].reshape(128, MB, npad)
        # blk[p, mb, t] = y[roffs[g]+t, mb*128+p]
        out[roffs[g] : roffs[g + 1]] = (
            blk[:, :, :n].transpose(2, 1, 0).reshape(n, HIDDEN)
        )
    return out
